# revision 5
# baseline (speedup 1.0000x reference)
"""MoE (63 routed experts top-7 + 1 shared expert) Trainium2 kernel.

Strategy: expert-parallel sparse dispatch with mixed precision. The router
(softmax + top-k, ~0.3% of FLOPs) runs on host; tokens are gathered
expert-major into fixed-capacity (1024-token) weight slots distributed
across 8 NeuronCores. Routed-expert slots run in fp8e4 with the PE's
DoubleRow perf mode (2 contraction k-tiles per instruction at 0.5
cycles/row = 4x fp16 matmul throughput); their quantization noise is
damped by the router gates (RMS gate weight ~0.1), keeping end-to-end
error ~3e-3. The shared expert - whose error enters at weight 1.0 - runs
in fp16 in its own slot (exactly 1024 tokens per core). Feature-major
layout throughout (features on partitions, tokens on the free dim):
weights need no transpose, biases ride the activation unit's per-partition
bias port, and the layer-2 scale+bias runs on the vector engine so the
scalar engine only ever evaluates Gelu (no activation-table reloads).
Outputs are gathered and gate-weighted back on host in the reference's
accumulation order.
"""

import os
import sys
import math

sys.path.insert(0, "/opt/trn_rl_repo")

import numpy as np

D = 1280          # model dim
I = 1280          # expert inter dim
EXPERTS = 63      # routed experts
TOPK = 7          # routed top-k
CAP = 1024        # tokens per weight slot
CHUNK = 512       # tokens per matmul / PSUM bank
KT = D // 128     # 10 contraction tiles
NCORES = 8
SW = 32.0         # fp8 weight pre-scale (power of 2; undone in act/DVE)

_PROGRAM_CACHE = {}


# ----------------------------------------------------------------- router

def _route(x2d, wr, br):
    """f32 softmax + top-k, matching jax.nn.softmax / jax.lax.top_k."""
    logits = (x2d @ wr + br).astype(np.float32)
    logits -= logits.max(-1, keepdims=True)
    np.exp(logits, out=logits)
    aff = logits / logits.sum(-1, keepdims=True)
    idx = np.argsort(-aff, axis=-1, kind="stable")[:, :TOPK]
    vals = np.take_along_axis(aff, idx, axis=-1)
    return idx.astype(np.int32), vals.astype(np.float32)


def _build_plan(T, idx):
    """Pack (token, expert) pairs expert-major into CAP-token routed pieces.
    Returns pieces padded with dummies to NR*NCORES, NR, the expert-major
    order, and the token id of each (token, k) pair."""
    flat = idx.ravel()
    order = np.argsort(flat, kind="stable")          # expert-major slot order
    tok_of = (order // TOPK).astype(np.int64)
    counts = np.bincount(flat, minlength=EXPERTS)
    offs = np.concatenate([[0], np.cumsum(counts)])

    pieces = []  # (expert, a, b)  [a:b) into the expert-major order
    for e in range(EXPERTS):
        a, b = int(offs[e]), int(offs[e + 1])
        while a < b:
            n = min(CAP, b - a)
            pieces.append((e, a, a + n))
            a += n

    NR = max(1, math.ceil(len(pieces) / NCORES))
    while len(pieces) < NR * NCORES:
        pieces.append((-1, 0, 0))                    # dummy slot
    return pieces, NR, order, tok_of


# ----------------------------------------------------------- device program

def _build_program(NR):
    import concourse.bass as bass
    import concourse.mybir as mybir
    import concourse.tile as tile
    from concourse import bacc

    f32 = mybir.dt.float32
    f16 = mybir.dt.float16
    f8 = mybir.dt.float8e4
    DR = mybir.MatmulPerfMode.DoubleRow
    Gelu = mybir.ActivationFunctionType.Gelu
    MR = NR * CAP

    nc = bacc.Bacc("TRN2", target_bir_lowering=False, debug=False,
                   enable_asserts=False, num_devices=NCORES)
    # partition-major DRAM layouts: [p, k, col] / [slot, p, io, ko, c]
    xr = nc.dram_tensor("xr", [128, KT, MR], f8, kind="ExternalInput").ap()
    xs = nc.dram_tensor("xs", [128, KT, CAP], f16, kind="ExternalInput").ap()
    w1r = nc.dram_tensor("w1r", [NR, 128, KT, KT, 128], f8, kind="ExternalInput").ap()
    w2r = nc.dram_tensor("w2r", [NR, 128, KT, KT, 128], f8, kind="ExternalInput").ap()
    b1r = nc.dram_tensor("b1r", [NR, 128, KT], f32, kind="ExternalInput").ap()
    b2r = nc.dram_tensor("b2r", [NR, 128, KT], f32, kind="ExternalInput").ap()
    ws1 = nc.dram_tensor("ws1", [KT, 128, KT, 128], f16, kind="ExternalInput").ap()
    ws2 = nc.dram_tensor("ws2", [KT, 128, KT, 128], f16, kind="ExternalInput").ap()
    sb1 = nc.dram_tensor("sb1", [128, KT], f32, kind="ExternalInput").ap()
    sb2 = nc.dram_tensor("sb2", [128, KT], f32, kind="ExternalInput").ap()
    yr = nc.dram_tensor("yr", [128, KT, MR], f8, kind="ExternalOutput").ap()
    ys = nc.dram_tensor("ys", [128, KT, CAP], f16, kind="ExternalOutput").ap()

    mult = mybir.AluOpType.mult
    add = mybir.AluOpType.add
    inv_sw = 1.0 / SW

    with tile.TileContext(nc) as tc:
        with (
            tc.tile_pool(name="xa", bufs=2) as xa,
            tc.tile_pool(name="wp", bufs=2) as wp,
            tc.tile_pool(name="swp", bufs=4) as swp,
            tc.tile_pool(name="hp", bufs=2) as hp,
            tc.tile_pool(name="yo", bufs=2) as yo,
            tc.tile_pool(name="bp", bufs=2) as bp,
            tc.tile_pool(name="ps", bufs=8, space="PSUM") as ps,
        ):
            # ---------------- routed fp8 DoubleRow slots ----------------
            for s in range(NR):
                b1t = bp.tile([128, KT], f32, tag="b1", name="b1t")
                nc.sync.dma_start(out=b1t[:, :], in_=b1r[s])
                b2t = bp.tile([128, KT], f32, tag="b2", name="b2t")
                nc.sync.dma_start(out=b2t[:, :], in_=b2r[s])

                xt = xa.tile([128, KT, CAP], f8, tag="x", name="xt")
                nc.gpsimd.dma_start(out=xt[:, :, :],
                                    in_=xr[:, :, s * CAP:(s + 1) * CAP])

                w1t = wp.tile([128, KT, KT, 128], f8, tag="w1", name="w1t")
                nc.sync.dma_start(out=w1t[:, :, :, :], in_=w1r[s])

                ht = hp.tile([128, KT, CAP], f8, tag="h", name="ht")
                # layer 1: h = gelu((x @ w1*SW)/SW + b1)
                for io in range(KT):
                    for c in range(CAP // CHUNK):
                        pt = ps.tile([128, CHUNK], f32, tag="ps", name="pt")
                        for kp in range(KT // 2):
                            nc.tensor.matmul(
                                pt[:, :],
                                w1t[:, io, 2 * kp:2 * kp + 2, :],
                                xt[:, 2 * kp:2 * kp + 2,
                                   c * CHUNK:(c + 1) * CHUNK],
                                start=(kp == 0), stop=(kp == KT // 2 - 1),
                                perf_mode=DR)
                        nc.scalar.activation(
                            ht[:, io, c * CHUNK:(c + 1) * CHUNK], pt[:, :],
                            Gelu, bias=b1t[:, io:io + 1], scale=inv_sw)

                w2t = wp.tile([128, KT, KT, 128], f8, tag="w2", name="w2t")
                nc.sync.dma_start(out=w2t[:, :, :, :], in_=w2r[s])

                yt = yo.tile([128, KT, CAP], f8, tag="y", name="yt")
                # layer 2: y = (h @ w2*SW)/SW + b2 (scale+bias on DVE)
                for io in range(KT):
                    for c in range(CAP // CHUNK):
                        pt = ps.tile([128, CHUNK], f32, tag="ps", name="pt")
                        for kp in range(KT // 2):
                            nc.tensor.matmul(
                                pt[:, :],
                                w2t[:, io, 2 * kp:2 * kp + 2, :],
                                ht[:, 2 * kp:2 * kp + 2,
                                   c * CHUNK:(c + 1) * CHUNK],
                                start=(kp == 0), stop=(kp == KT // 2 - 1),
                                perf_mode=DR)
                        nc.vector.tensor_scalar(
                            yt[:, io, c * CHUNK:(c + 1) * CHUNK], pt[:, :],
                            inv_sw, b2t[:, io:io + 1], mult, add)
                nc.sync.dma_start(out=yr[:, :, s * CAP:(s + 1) * CAP],
                                  in_=yt[:, :, :])

            # ------------------- shared fp16 slot -----------------------
            sb1t = bp.tile([128, KT], f32, tag="b1", name="sb1t")
            nc.sync.dma_start(out=sb1t[:, :], in_=sb1)
            sb2t = bp.tile([128, KT], f32, tag="b2", name="sb2t")
            nc.sync.dma_start(out=sb2t[:, :], in_=sb2)

            xst = xa.tile([128, KT, CAP], f16, tag="xs", name="xst", bufs=1)
            nc.gpsimd.dma_start(out=xst[:, :, :], in_=xs[:, :, :])

            hst = hp.tile([128, KT, CAP], f16, tag="hs", name="hst", bufs=1)
            for io in range(KT):
                wt = swp.tile([128, KT, 128], f16, tag="sw", name="sw1t")
                nc.sync.dma_start(out=wt[:, :, :], in_=ws1[io])
                for c in range(CAP // CHUNK):
                    pt = ps.tile([128, CHUNK], f32, tag="ps", name="pt")
                    for k in range(KT):
                        nc.tensor.matmul(
                            pt[:, :], wt[:, k, :],
                            xst[:, k, c * CHUNK:(c + 1) * CHUNK],
                            start=(k == 0), stop=(k == KT - 1))
                    nc.scalar.activation(
                        hst[:, io, c * CHUNK:(c + 1) * CHUNK], pt[:, :],
                        Gelu, bias=sb1t[:, io:io + 1])

            yst = yo.tile([128, KT, CAP], f16, tag="ys", name="yst", bufs=1)
            for io in range(KT):
                wt = swp.tile([128, KT, 128], f16, tag="sw", name="sw2t")
                nc.sync.dma_start(out=wt[:, :, :], in_=ws2[io])
                for c in range(CAP // CHUNK):
                    pt = ps.tile([128, CHUNK], f32, tag="ps", name="pt")
                    for k in range(KT):
                        nc.tensor.matmul(
                            pt[:, :], wt[:, k, :],
                            hst[:, k, c * CHUNK:(c + 1) * CHUNK],
                            start=(k == 0), stop=(k == KT - 1))
                    nc.vector.tensor_scalar(
                        yst[:, io, c * CHUNK:(c + 1) * CHUNK], pt[:, :],
                        1.0, sb2t[:, io:io + 1], mult, add)
            nc.sync.dma_start(out=ys[:, :, :], in_=yst[:, :, :])
    nc.compile()
    return nc


def _get_program(NR):
    if NR not in _PROGRAM_CACHE:
        _PROGRAM_CACHE[NR] = _build_program(NR)
    return _PROGRAM_CACHE[NR]


# ------------------------------------------------------------------ kernel

def _f8():
    import ml_dtypes
    return ml_dtypes.float8_e4m3


def _arrange_w(w):
    """[D, I] -> [p, io, ko, c] so a whole slot-layer DMAs in one start
    into an SBUF tile laid out [partition, io, ko, col]."""
    return np.ascontiguousarray(
        w.reshape(KT, 128, KT, 128).transpose(1, 2, 0, 3))


def _arrange_w_io(w):
    """[D, I] -> [io, p, ko, c] (per-io tiles, shared-expert path)."""
    return np.ascontiguousarray(
        w.reshape(KT, 128, KT, 128).transpose(2, 1, 0, 3))


def kernel(x, sw1, sb1, sw2, sb2, rw1, rb1, rw2, rb2, wr, br, _trace=False):
    from concourse.bass_utils import run_bass_kernel_spmd

    f8 = _f8()
    x = np.asarray(x, dtype=np.float32)
    B, Sq, _ = x.shape
    T = B * Sq
    assert T == NCORES * CAP
    xf = np.ascontiguousarray(x.reshape(T, D))

    idx, vals = _route(xf, np.asarray(wr, np.float32), np.asarray(br, np.float32))
    pieces, NR, order, tok_of = _build_plan(T, idx)
    MR = NR * CAP

    rw1 = np.asarray(rw1, np.float32); rw2 = np.asarray(rw2, np.float32)
    rb1 = np.asarray(rb1, np.float32); rb2 = np.asarray(rb2, np.float32)
    sw1 = np.asarray(sw1, np.float32); sw2 = np.asarray(sw2, np.float32)
    sb1 = np.asarray(sb1, np.float32); sb2 = np.asarray(sb2, np.float32)

    w1a = [(_arrange_w(rw1[e]) * SW).astype(f8) for e in range(EXPERTS)]
    w2a = [(_arrange_w(rw2[e]) * SW).astype(f8) for e in range(EXPERTS)]
    b1a = [np.ascontiguousarray(rb1[e].reshape(KT, 128).T) for e in range(EXPERTS)]
    b2a = [np.ascontiguousarray(rb2[e].reshape(KT, 128).T) for e in range(EXPERTS)]
    sw1a = _arrange_w_io(sw1).astype(np.float16)
    sw2a = _arrange_w_io(sw2).astype(np.float16)
    sb1a = np.ascontiguousarray(sb1.reshape(KT, 128).T)
    sb2a = np.ascontiguousarray(sb2.reshape(KT, 128).T)

    # [p, k, token] views of x, fp8 (routed) and fp16 (shared)
    xpk8 = np.ascontiguousarray(
        xf.T.reshape(KT, 128, T).transpose(1, 0, 2)).astype(f8)
    xpk16 = np.ascontiguousarray(
        xf.T.reshape(KT, 128, T).transpose(1, 0, 2)).astype(np.float16)

    in_maps = []
    for core in range(NCORES):
        xr_core = np.zeros((128, KT, MR), dtype=f8)
        w1_core = np.zeros((NR, 128, KT, KT, 128), dtype=f8)
        w2_core = np.zeros((NR, 128, KT, KT, 128), dtype=f8)
        b1_core = np.zeros((NR, 128, KT), dtype=np.float32)
        b2_core = np.zeros((NR, 128, KT), dtype=np.float32)
        for j in range(NR):
            e, a, b = pieces[core * NR + j]
            if e < 0:
                continue
            toks = tok_of[a:b]
            xr_core[:, :, j * CAP: j * CAP + (b - a)] = xpk8[:, :, toks]
            w1_core[j] = w1a[e]; w2_core[j] = w2a[e]
            b1_core[j] = b1a[e]; b2_core[j] = b2a[e]
        in_maps.append({
            "xr": xr_core,
            "xs": np.ascontiguousarray(
                xpk16[:, :, core * CAP:(core + 1) * CAP]),
            "w1r": w1_core, "w2r": w2_core,
            "b1r": b1_core, "b2r": b2_core,
            "ws1": sw1a, "ws2": sw2a,
            "sb1": sb1a, "sb2": sb2a,
        })

    nc = _get_program(NR)
    res = run_bass_kernel_spmd(nc, in_maps, core_ids=list(range(NCORES)),
                               trace=_trace)
    kernel.last_result = res

    TK = T * TOPK
    gated = np.empty((TK, D), dtype=np.float32)   # expert-major rows
    shared_out = np.empty((T, D), dtype=np.float32)
    for core in range(NCORES):
        Y = res.results[core]["yr"]               # [128, KT, MR] fp8
        Yf = np.asarray(Y, dtype=np.float32).transpose(1, 0, 2).reshape(D, MR)
        for j in range(NR):
            e, a, b = pieces[core * NR + j]
            if e < 0 or a == b:
                continue
            gated[a:b] = Yf[:, j * CAP: j * CAP + (b - a)].T
        Ys = np.asarray(res.results[core]["ys"], dtype=np.float32)
        shared_out[core * CAP:(core + 1) * CAP] = \
            Ys.transpose(1, 0, 2).reshape(D, CAP).T

    g = vals.ravel()[order].astype(np.float32)
    gated *= g[:, None]
    ord2 = np.argsort(tok_of, kind="stable")      # token-major, expert asc
    routed = gated[ord2].reshape(T, TOPK, D).sum(axis=1, dtype=np.float32)

    out = shared_out + routed + xf
    return out.reshape(B, Sq, D).astype(np.float32)


kernel.last_result = None


# revision 10
# speedup vs baseline: 1.1003x; 1.1003x over previous
"""MoE (63 routed experts top-7 + 1 shared expert) Trainium2 kernel.

Strategy: expert-parallel sparse dispatch with mixed precision. The router
(softmax + top-k, ~0.3% of FLOPs) runs on host; tokens are gathered
expert-major into variable-size weight slots (sized to the actual expert
loads, so ~9% less PE work than fixed 1024-capacity slots) distributed
across 8 NeuronCores. Routed-expert slots run in fp8e4 with the PE's
DoubleRow perf mode (2 contraction k-tiles per instruction = 2x fp16
matmul throughput on TRN2); their quantization noise is damped by the
router gates (RMS gate weight ~0.1). The shared expert - whose error
enters at weight 1.0 - runs entirely in fp16 (fp8 there costs 4x the
error for a 5% speedup; not worth the margin).
Feature-major layout throughout: weights need no transpose, biases
ride the activation unit's per-partition bias port, and layer-2 scale+bias
runs on the vector engine so the scalar engine only ever evaluates Gelu
(no activation-table reloads). The shared slot is emitted mid-program so
the kernel tail is a cheap fp8 slot drain. Outputs are gathered and
gate-weighted back on host in the reference's accumulation order.
"""

import sys
import math

sys.path.insert(0, "/opt/trn_rl_repo")

import numpy as np

D = 1280          # model dim
I = 1280          # expert inter dim
EXPERTS = 63      # routed experts
TOPK = 7          # routed top-k
CAP = 1024        # max tokens per weight slot
CHUNK = 512       # max tokens per matmul / PSUM bank
KT = D // 128     # 10 contraction tiles
NCORES = 8
SW = 32.0         # fp8 weight pre-scale (power of 2; undone in act/DVE)

_PROGRAM_CACHE = {}


def _chunks_of(sz):
    n = max(1, math.ceil(sz / CHUNK))
    base = sz // n
    c0 = min(sz, (base + 15) // 16 * 16) if n > 1 else sz
    out = []
    left = sz
    for i in range(n - 1):
        out.append(c0)
        left -= c0
    out.append(left)
    assert sum(out) == sz and all(0 < c <= CHUNK for c in out)
    return out


# ----------------------------------------------------------------- router

def _route(x2d, wr, br):
    """f32 softmax + top-k, matching jax.nn.softmax / jax.lax.top_k."""
    logits = (x2d @ wr + br).astype(np.float32)
    logits -= logits.max(-1, keepdims=True)
    np.exp(logits, out=logits)
    aff = logits / logits.sum(-1, keepdims=True)
    idx = np.argsort(-aff, axis=-1, kind="stable")[:, :TOPK]
    vals = np.take_along_axis(aff, idx, axis=-1)
    return idx.astype(np.int32), vals.astype(np.float32)


def _build_plan(T, idx):
    """Pack (token, expert) pairs expert-major into <=CAP-token routed
    pieces, sort by size, and form NR slot groups of NCORES pieces each so
    every slot index has near-equal sizes across cores. Returns the
    (core, slot) -> piece assignment, per-slot sizes, the expert-major
    order, and the token of each (token, k) pair."""
    flat = idx.ravel()
    order = np.argsort(flat, kind="stable")          # expert-major slot order
    tok_of = (order // TOPK).astype(np.int64)
    counts = np.bincount(flat, minlength=EXPERTS)
    offs = np.concatenate([[0], np.cumsum(counts)])

    pieces = []  # (expert, a, b)  [a:b) into the expert-major order
    for e in range(EXPERTS):
        a, b = int(offs[e]), int(offs[e + 1])
        while a < b:
            n = min(CAP, b - a)
            pieces.append((e, a, a + n))
            a += n

    NR = max(1, math.ceil(len(pieces) / NCORES))
    while len(pieces) < NR * NCORES:
        pieces.append((-1, 0, 0))                    # dummy
    pieces.sort(key=lambda p: p[1] - p[2])           # size desc
    assign = {}                                      # (core, slot) -> piece
    slot_sizes = []
    for j in range(NR):
        grp = pieces[j * NCORES:(j + 1) * NCORES]
        sz = max(16, (max(b - a for _, a, b in grp) + 15) // 16 * 16)
        slot_sizes.append(sz)
        for c, p in enumerate(grp):
            assign[(c, j)] = p
    return assign, tuple(slot_sizes), order, tok_of


# ----------------------------------------------------------- device program

def _build_program(slot_sizes):
    import concourse.mybir as mybir
    import concourse.tile as tile
    from concourse import bacc

    f32 = mybir.dt.float32
    f16 = mybir.dt.float16
    f8 = mybir.dt.float8e4
    DR = mybir.MatmulPerfMode.DoubleRow
    Gelu = mybir.ActivationFunctionType.Gelu
    mult = mybir.AluOpType.mult
    add = mybir.AluOpType.add
    inv_sw = 1.0 / SW

    NR = len(slot_sizes)
    offs = [0]
    for sz in slot_sizes:
        offs.append(offs[-1] + sz)
    MR = offs[-1]
    SHPOS = NR // 2                                  # emit shared slot here

    nc = bacc.Bacc("TRN2", target_bir_lowering=False, debug=False,
                   enable_asserts=False, num_devices=NCORES)
    # partition-major DRAM layouts: [p, k, col] / [slot, p, io, ko, c]
    xr = nc.dram_tensor("xr", [128, KT, MR], f8, kind="ExternalInput").ap()
    xs = nc.dram_tensor("xs", [128, KT, CAP], f16, kind="ExternalInput").ap()
    w1r = nc.dram_tensor("w1r", [NR, 128, KT, KT, 128], f8, kind="ExternalInput").ap()
    w2r = nc.dram_tensor("w2r", [NR, 128, KT, KT, 128], f8, kind="ExternalInput").ap()
    b1r = nc.dram_tensor("b1r", [NR, 128, KT], f32, kind="ExternalInput").ap()
    b2r = nc.dram_tensor("b2r", [NR, 128, KT], f32, kind="ExternalInput").ap()
    ws1 = nc.dram_tensor("ws1", [KT, 128, KT, 128], f16, kind="ExternalInput").ap()
    ws2 = nc.dram_tensor("ws2", [KT, 128, KT, 128], f16, kind="ExternalInput").ap()
    sb1 = nc.dram_tensor("sb1", [128, KT], f32, kind="ExternalInput").ap()
    sb2 = nc.dram_tensor("sb2", [128, KT], f32, kind="ExternalInput").ap()
    yr = nc.dram_tensor("yr", [128, KT, MR], f8, kind="ExternalOutput").ap()
    ys = nc.dram_tensor("ys", [128, KT, CAP], f16, kind="ExternalOutput").ap()

    with tile.TileContext(nc) as tc:
        with (
            tc.tile_pool(name="xa", bufs=2) as xa,
            tc.tile_pool(name="wp", bufs=2) as wp,
            tc.tile_pool(name="swp", bufs=4) as swp,
            tc.tile_pool(name="hp", bufs=2) as hp,
            tc.tile_pool(name="yo", bufs=2) as yo,
            tc.tile_pool(name="bp", bufs=2) as bp,
            tc.tile_pool(name="ps", bufs=8, space="PSUM") as ps,
        ):
            def routed_slot(s):
                sz = slot_sizes[s]
                chunks = _chunks_of(sz)
                col0 = offs[s]
                b1t = bp.tile([128, KT], f32, tag="b1", name="b1t")
                nc.sync.dma_start(out=b1t[:, :], in_=b1r[s])
                b2t = bp.tile([128, KT], f32, tag="b2", name="b2t")
                nc.sync.dma_start(out=b2t[:, :], in_=b2r[s])

                xt = xa.tile([128, KT, CAP], f8, tag="x", name="xt")
                c0 = 0
                for ch in chunks:
                    nc.gpsimd.dma_start(
                        out=xt[:, :, c0:c0 + ch],
                        in_=xr[:, :, col0 + c0:col0 + c0 + ch])
                    c0 += ch

                w1t = wp.tile([128, KT, KT, 128], f8, tag="w1", name="w1t")
                for io2 in range(0, KT, 2):
                    nc.sync.dma_start(out=w1t[:, io2:io2 + 2, :, :],
                                      in_=w1r[s, :, io2:io2 + 2])

                ht = hp.tile([128, KT, CAP], f8, tag="h", name="ht")
                # layer 1: h = gelu((x @ w1*SW)/SW + b1)
                for io in range(KT):
                    c0 = 0
                    for ch in chunks:
                        pt = ps.tile([128, CHUNK], f32, tag="ps", name="pt")
                        for kp in range(KT // 2):
                            nc.tensor.matmul(
                                pt[:, :ch],
                                w1t[:, io, 2 * kp:2 * kp + 2, :],
                                xt[:, 2 * kp:2 * kp + 2, c0:c0 + ch],
                                start=(kp == 0), stop=(kp == KT // 2 - 1),
                                perf_mode=DR)
                        nc.scalar.activation(
                            ht[:, io, c0:c0 + ch], pt[:, :ch],
                            Gelu, bias=b1t[:, io:io + 1], scale=inv_sw)
                        c0 += ch

                w2t = wp.tile([128, KT, KT, 128], f8, tag="w2", name="w2t")
                for io2 in range(0, KT, 2):
                    nc.sync.dma_start(out=w2t[:, io2:io2 + 2, :, :],
                                      in_=w2r[s, :, io2:io2 + 2])

                yt = yo.tile([128, KT, CAP], f8, tag="y", name="yt")
                # layer 2: y = (h @ w2*SW)/SW + b2 (scale+bias on DVE)
                for io in range(KT):
                    c0 = 0
                    for ch in chunks:
                        pt = ps.tile([128, CHUNK], f32, tag="ps", name="pt")
                        for kp in range(KT // 2):
                            nc.tensor.matmul(
                                pt[:, :ch],
                                w2t[:, io, 2 * kp:2 * kp + 2, :],
                                ht[:, 2 * kp:2 * kp + 2, c0:c0 + ch],
                                start=(kp == 0), stop=(kp == KT // 2 - 1),
                                perf_mode=DR)
                        nc.vector.tensor_scalar(
                            yt[:, io, c0:c0 + ch], pt[:, :ch],
                            inv_sw, b2t[:, io:io + 1], mult, add)
                        c0 += ch
                c0 = 0
                for ch in chunks:
                    nc.sync.dma_start(
                        out=yr[:, :, col0 + c0:col0 + c0 + ch],
                        in_=yt[:, :, c0:c0 + ch])
                    c0 += ch

            def shared_slot():
                sb1t = bp.tile([128, KT], f32, tag="b1", name="sb1t")
                nc.sync.dma_start(out=sb1t[:, :], in_=sb1)
                sb2t = bp.tile([128, KT], f32, tag="b2", name="sb2t")
                nc.sync.dma_start(out=sb2t[:, :], in_=sb2)

                xst = xa.tile([128, KT, CAP], f16, tag="xs", name="xst", bufs=1)
                for c0 in range(0, CAP, CHUNK):
                    nc.gpsimd.dma_start(out=xst[:, :, c0:c0 + CHUNK],
                                        in_=xs[:, :, c0:c0 + CHUNK])

                # entire shared expert in fp16: its error is undamped by gates
                hst = hp.tile([128, KT, CAP], f16, tag="hs", name="hst", bufs=1)
                for io in range(KT):
                    wt = swp.tile([128, KT, 128], f16, tag="sw", name="sw1t")
                    nc.sync.dma_start(out=wt[:, :, :], in_=ws1[io])
                    for c0 in range(0, CAP, CHUNK):
                        pt = ps.tile([128, CHUNK], f32, tag="ps", name="pt")
                        for k in range(KT):
                            nc.tensor.matmul(
                                pt[:, :], wt[:, k, :],
                                xst[:, k, c0:c0 + CHUNK],
                                start=(k == 0), stop=(k == KT - 1))
                        nc.scalar.activation(
                            hst[:, io, c0:c0 + CHUNK], pt[:, :],
                            Gelu, bias=sb1t[:, io:io + 1])

                yst = yo.tile([128, KT, CAP], f16, tag="ys", name="yst", bufs=1)
                for io in range(KT):
                    wt = swp.tile([128, KT, 128], f16, tag="sw", name="sw2t")
                    nc.sync.dma_start(out=wt[:, :, :], in_=ws2[io])
                    for c0 in range(0, CAP, CHUNK):
                        pt = ps.tile([128, CHUNK], f32, tag="ps", name="pt")
                        for k in range(KT):
                            nc.tensor.matmul(
                                pt[:, :], wt[:, k, :],
                                hst[:, k, c0:c0 + CHUNK],
                                start=(k == 0), stop=(k == KT - 1))
                        nc.vector.tensor_scalar(
                            yst[:, io, c0:c0 + CHUNK], pt[:, :],
                            1.0, sb2t[:, io:io + 1], mult, add)
                for c0 in range(0, CAP, CHUNK):
                    nc.sync.dma_start(out=ys[:, :, c0:c0 + CHUNK],
                                      in_=yst[:, :, c0:c0 + CHUNK])

            for s in range(NR):
                if s == SHPOS:
                    shared_slot()
                routed_slot(s)
    nc.compile()
    return nc


def _get_program(slot_sizes):
    if slot_sizes not in _PROGRAM_CACHE:
        _PROGRAM_CACHE[slot_sizes] = _build_program(slot_sizes)
    return _PROGRAM_CACHE[slot_sizes]


# ------------------------------------------------------------------ kernel

def _f8():
    import ml_dtypes
    return ml_dtypes.float8_e4m3


def _arrange_w(w):
    """[D, I] -> [p, io, ko, c] so a whole slot-layer DMAs contiguously
    into an SBUF tile laid out [partition, io, ko, col]."""
    return np.ascontiguousarray(
        w.reshape(KT, 128, KT, 128).transpose(1, 2, 0, 3))


def _arrange_w_io(w):
    """[D, I] -> [io, p, ko, c] (per-io tiles, shared-expert L1 path)."""
    return np.ascontiguousarray(
        w.reshape(KT, 128, KT, 128).transpose(2, 1, 0, 3))


def kernel(x, sw1, sb1, sw2, sb2, rw1, rb1, rw2, rb2, wr, br, _trace=False):
    from concourse.bass_utils import run_bass_kernel_spmd

    f8 = _f8()
    x = np.asarray(x, dtype=np.float32)
    B, Sq, _ = x.shape
    T = B * Sq
    assert T == NCORES * CAP
    xf = np.ascontiguousarray(x.reshape(T, D))

    idx, vals = _route(xf, np.asarray(wr, np.float32), np.asarray(br, np.float32))
    assign, slot_sizes, order, tok_of = _build_plan(T, idx)
    NR = len(slot_sizes)
    offs = [0]
    for sz in slot_sizes:
        offs.append(offs[-1] + sz)
    MR = offs[-1]

    rw1 = np.asarray(rw1, np.float32); rw2 = np.asarray(rw2, np.float32)
    rb1 = np.asarray(rb1, np.float32); rb2 = np.asarray(rb2, np.float32)
    sw1 = np.asarray(sw1, np.float32); sw2 = np.asarray(sw2, np.float32)
    sb1 = np.asarray(sb1, np.float32); sb2 = np.asarray(sb2, np.float32)

    w1a = [(_arrange_w(rw1[e]) * SW).astype(f8) for e in range(EXPERTS)]
    w2a = [(_arrange_w(rw2[e]) * SW).astype(f8) for e in range(EXPERTS)]
    b1a = [np.ascontiguousarray(rb1[e].reshape(KT, 128).T) for e in range(EXPERTS)]
    b2a = [np.ascontiguousarray(rb2[e].reshape(KT, 128).T) for e in range(EXPERTS)]
    sw1a = _arrange_w_io(sw1).astype(np.float16)
    sw2a = _arrange_w_io(sw2).astype(np.float16)
    sb1a = np.ascontiguousarray(sb1.reshape(KT, 128).T)
    sb2a = np.ascontiguousarray(sb2.reshape(KT, 128).T)

    # [p, k, token] views of x, fp8 (routed) and fp16 (shared)
    xpk = np.ascontiguousarray(xf.T.reshape(KT, 128, T).transpose(1, 0, 2))
    xpk8 = xpk.astype(f8)
    xpk16 = xpk.astype(np.float16)

    in_maps = []
    for core in range(NCORES):
        xr_core = np.zeros((128, KT, MR), dtype=f8)
        w1_core = np.zeros((NR, 128, KT, KT, 128), dtype=f8)
        w2_core = np.zeros((NR, 128, KT, KT, 128), dtype=f8)
        b1_core = np.zeros((NR, 128, KT), dtype=np.float32)
        b2_core = np.zeros((NR, 128, KT), dtype=np.float32)
        for j in range(NR):
            e, a, b = assign[(core, j)]
            if e < 0 or a == b:
                continue
            toks = tok_of[a:b]
            xr_core[:, :, offs[j]: offs[j] + (b - a)] = xpk8[:, :, toks]
            w1_core[j] = w1a[e]; w2_core[j] = w2a[e]
            b1_core[j] = b1a[e]; b2_core[j] = b2a[e]
        in_maps.append({
            "xr": xr_core,
            "xs": np.ascontiguousarray(
                xpk16[:, :, core * CAP:(core + 1) * CAP]),
            "w1r": w1_core, "w2r": w2_core,
            "b1r": b1_core, "b2r": b2_core,
            "ws1": sw1a, "ws2": sw2a,
            "sb1": sb1a, "sb2": sb2a,
        })

    nc = _get_program(slot_sizes)
    res = run_bass_kernel_spmd(nc, in_maps, core_ids=list(range(NCORES)),
                               trace=_trace)
    kernel.last_result = res

    TK = T * TOPK
    gated = np.empty((TK, D), dtype=np.float32)   # expert-major rows
    shared_out = np.empty((T, D), dtype=np.float32)
    for core in range(NCORES):
        Y = res.results[core]["yr"]               # [128, KT, MR] fp8
        Yf = np.asarray(Y, dtype=np.float32).transpose(1, 0, 2).reshape(D, MR)
        for j in range(NR):
            e, a, b = assign[(core, j)]
            if e < 0 or a == b:
                continue
            gated[a:b] = Yf[:, offs[j]: offs[j] + (b - a)].T
        Ys = np.asarray(res.results[core]["ys"], dtype=np.float32)
        shared_out[core * CAP:(core + 1) * CAP] = \
            Ys.transpose(1, 0, 2).reshape(D, CAP).T

    g = vals.ravel()[order].astype(np.float32)
    gated *= g[:, None]
    ord2 = np.argsort(tok_of, kind="stable")      # token-major, expert asc
    routed = gated[ord2].reshape(T, TOPK, D).sum(axis=1, dtype=np.float32)

    out = shared_out + routed + xf
    return out.reshape(B, Sq, D).astype(np.float32)


kernel.last_result = None
